# revision 24
# baseline (speedup 1.0000x reference)
"""MoE 2D router kernel for 8 Trainium2 NeuronCores — v5.

Strategy (pure data parallel, batch-sharded):
  - B=16 batches split across 8 cores (2 per core). Per core, each batch's
    [C=16, H=128, W=128] tensor is viewed as [128, 2048] with partition
    p = c*8 + blk (blk = pixel-block of 2048 contiguous pixels); HBM loads
    are fully contiguous.
  - Gates are computed in A-space where the channel params are
    per-partition scalars, so the parameter multiplies ride the ACT
    engine's scale input for free: wg = Copy(x; scale=wgp),
    wnoise = Ln(1 + Exp(x; scale=wnp)), nw = noise * wnoise.
  - hl = wg + nw is fused into the PE: two accumulating f32 transposes
    into one PSUM tile produce hl directly in pixel-major (T) space,
    bit-identical to an elementwise add. nw and wnoise are transposed
    alongside.
  - In T-space the expert axis c sits on the free axis (stride 8):
      * top-1 / masked top-2 over experts are strided free-axis reduces,
      * per-pixel stats broadcast back over c as stride-0 views,
      * the softmax denominator is a free-axis add-reduce,
      * G = mask * bcast(1/sum_c exp(hl_c - m1)) via the already-computed
        u = hl - m1 (shift-normalized softmax; argmax term is exactly 1),
      * the numerator uses u = hl - m1 (exact at the argmax):
        wg - max_excl = u - mask*(m2-m1) - nw.
  - softplus on the combined exp/ln table; 1/wnoise = Exp(-Ln(wnoise)) on
    the same table; load = Erf(q) from the erf table, executed in two
    groups with explicit ACT-queue ordering edges: 4 table loads total.
  - Outputs are written in transposed layout; the host inverts the
    permutation while unsharding.
"""
import sys

sys.path.insert(0, "/opt/trn_rl_repo")

import numpy as np

B, C, H, W = 16, 16, 128, 128
NCORES = 8
BPC = B // NCORES           # batches per core
HW = H * W                  # 16384 pixels per (batch, channel)
NBLK = 8                    # pixel blocks per batch (HW / 2048)
FB = C * HW // 128          # free size per batch in [128, FB] layout = 2048
CHW = 512                   # chunk width
NCH = CHW // 128            # 128-col transpose groups per chunk = 4
CPB = FB // CHW             # chunks per batch = 4
NCHUNK = BPC * CPB          # chunks per core = 8

_CACHE = {}


def _build():
    import concourse.bacc as bacc
    import concourse.mybir as mybir
    from concourse.tile import TileContext, add_dep_helper

    f32 = mybir.dt.float32
    bf16 = mybir.dt.bfloat16
    AX = mybir.AxisListType
    OP = mybir.AluOpType
    AF = mybir.ActivationFunctionType
    BIGNEG = -1e30

    nc = bacc.Bacc(trn_type="TRN2", target_bir_lowering=False, debug=False,
                   num_devices=NCORES, name="moe_router")

    xd = nc.dram_tensor("x", [BPC, 128, FB], f32, kind="ExternalInput")
    nd = nc.dram_tensor("noise", [BPC, 128, FB], f32, kind="ExternalInput")
    idf_d = nc.dram_tensor("id_f", [128, 128], f32, kind="ExternalInput")
    wgp_d = nc.dram_tensor("wgp", [128, 1], f32, kind="ExternalInput")
    wnp_d = nc.dram_tensor("wnp", [128, 1], f32, kind="ExternalInput")
    gd = nc.dram_tensor("g_out", [BPC, CPB, 128, CHW], f32,
                        kind="ExternalOutput")
    ld = nc.dram_tensor("load_out", [BPC, CPB, 128, CHW], f32,
                        kind="ExternalOutput")

    with TileContext(nc) as tc:
        with tc.tile_pool(name="const", bufs=1) as cpool, \
             tc.tile_pool(name="io", bufs=3) as iop, \
             tc.tile_pool(name="work", bufs=3) as wp, \
             tc.tile_pool(name="small", bufs=3) as sp, \
             tc.tile_pool(name="erf", bufs=1) as ep, \
             tc.tile_pool(name="ps_t", bufs=2, space="PSUM") as ps_t:

            consts = [None]

            def _load_consts():
                idf = cpool.tile([128, 128], f32, tag="idf")
                nc.sync.dma_start(out=idf[:, :], in_=idf_d[:, :])
                wgp = cpool.tile([128, 1], f32, tag="wgp")
                nc.sync.dma_start(out=wgp[:, :], in_=wgp_d[:, :])
                wnp = cpool.tile([128, 1], f32, tag="wnp")
                nc.sync.dma_start(out=wnp[:, :], in_=wnp_d[:, :])
                return idf, wgp, wnp

            qts = []
            t6_by_half = [[], []]  # table-6 ACT instructions per kernel half

            def _emit_erf_group(group, after_insts):
                first = None
                prev = None
                for bb, ch, qt in group:
                    lt = iop.tile([128, CHW], f32, tag=f"load{ch % 2}")
                    i = nc.scalar.activation(lt[:, :], qt[:, :], AF.Erf)
                    if first is None:
                        first = i
                        for t6 in after_insts:
                            # add_dep_helper(a, b) == "a waits for b"
                            add_dep_helper(i.ins, t6.ins, sync=True,
                                           reason="erf after exp/ln group")
                    else:
                        add_dep_helper(i.ins, prev.ins, sync=True,
                                       reason="erf chain")
                    prev = i
                    nc.gpsimd.dma_start(out=ld[bb, ch, :, :], in_=lt[:, :])
                return first, prev

            erf_a_last = None

            for chunk in range(NCHUNK):
                bb, ch = divmod(chunk, CPB)
                cs = ch * CHW
                half = chunk // (NCHUNK // 2)
                t6 = t6_by_half[half]

                # ---- load inputs ----
                xa = iop.tile([128, CHW], f32, tag="x")
                nc.sync.dma_start(out=xa[:, :], in_=xd[bb, :, cs:cs + CHW])
                na = iop.tile([128, CHW], f32, tag="noise")
                nc.sync.dma_start(out=na[:, :], in_=nd[bb, :, cs:cs + CHW])
                if consts[0] is None:
                    consts[0] = _load_consts()
                idf, wgp, wnp = consts[0]

                # ---- gates in A-space (params are per-partition scalars) ----
                wga = wp.tile([128, CHW], f32, tag="wga")
                t6.append(nc.scalar.activation(wga[:, :], xa[:, :], AF.Copy,
                                               scale=wgp[:, :]))
                eu0 = wp.tile([128, CHW], f32, tag="eu0")
                t6.append(nc.scalar.activation(eu0[:, :], xa[:, :], AF.Exp,
                                               scale=wnp[:, :]))
                wn = wp.tile([128, CHW], f32, tag="wn")
                t6.append(nc.scalar.activation(wn[:, :], eu0[:, :], AF.Ln,
                                               bias=1.0))
                nwa = wp.tile([128, CHW], f32, tag="nwa")
                nc.gpsimd.tensor_tensor(nwa[:, :], na[:, :], wn[:, :],
                                        op=OP.mult)

                # ---- PE transposes; hl = T(wg) + T(nw) via PSUM accum ----
                hlT = ps_t.tile([128, CHW], f32, tag="hlT")
                nwT = ps_t.tile([128, CHW], f32, tag="nwT")
                wnT = ps_t.tile([128, CHW], f32, tag="wnT")
                for g in range(NCH):
                    s = slice(g * 128, (g + 1) * 128)
                    nc.tensor.matmul(hlT[:, s], wga[:, s], idf[:, :],
                                     is_transpose=True, start=True, stop=False)
                    nc.tensor.matmul(hlT[:, s], nwa[:, s], idf[:, :],
                                     is_transpose=True, start=False, stop=True)
                    nc.tensor.transpose(nwT[:, s], nwa[:, s], idf[:, :])
                    nc.tensor.transpose(wnT[:, s], wn[:, s], idf[:, :])

                # ---- T-space activations ----
                lwT = wp.tile([128, CHW], f32, tag="lwT")
                t6.append(nc.scalar.activation(lwT[:, :], wnT[:, :], AF.Ln))
                rwT = wp.tile([128, CHW], f32, tag="rwT")
                t6.append(nc.scalar.activation(rwT[:, :], lwT[:, :], AF.Exp,
                                               scale=-1.0))
                # ---- expert-axis stats (strided free-axis reduces) ----
                vh = hlT[:, :].rearrange("p (g c k) -> p g k c", g=NCH, c=C)
                m1c = sp.tile([128, 32], f32, tag="m1c")
                nc.vector.tensor_reduce(m1c[:, :], vh, axis=AX.X, op=OP.max)
                m1b = (m1c[:, :].rearrange("p (g k) -> p g k", g=NCH)
                       .unsqueeze(2).broadcast_to([128, NCH, C, NBLK]))
                u = wp.tile([128, CHW], f32, tag="u")
                nc.vector.tensor_tensor(u[:, :], hlT[:, :], m1b, op=OP.subtract)
                un = wp.tile([128, CHW], f32, tag="un")
                nc.vector.tensor_tensor(un[:, :], u[:, :], nwT[:, :],
                                        op=OP.subtract)
                mk = wp.tile([128, CHW], bf16, tag="mk")
                nc.vector.tensor_scalar(mk[:, :], u[:, :], 0.0, None,
                                        op0=OP.is_equal)
                md = wp.tile([128, CHW], f32, tag="md")
                nc.vector.scalar_tensor_tensor(md[:, :], mk[:, :], BIGNEG,
                                               u[:, :], op0=OP.mult, op1=OP.add)
                eu = wp.tile([128, CHW], bf16, tag="eu")
                t6.append(nc.scalar.activation(eu[:, :], u[:, :], AF.Exp))
                vm = md[:, :].rearrange("p (g c k) -> p g k c", g=NCH, c=C)
                s2c = sp.tile([128, 32], f32, tag="s2c")
                nc.vector.tensor_reduce(s2c[:, :], vm, axis=AX.X, op=OP.max)
                ve = eu[:, :].rearrange("p (g c k) -> p g k c", g=NCH, c=C)
                ssc = sp.tile([128, 32], f32, tag="ssc")
                nc.vector.tensor_reduce(ssc[:, :], ve, axis=AX.X, op=OP.add)

                # ---- G = mask * bcast(1/sum_c exp(hl_c - m1)) ----
                g1c = sp.tile([128, 32], f32, tag="g1c")
                nc.vector.reciprocal(g1c[:, :], ssc[:, :])
                g1b = (g1c[:, :].rearrange("p (g k) -> p g k", g=NCH)
                       .unsqueeze(2).broadcast_to([128, NCH, C, NBLK]))
                gt = iop.tile([128, CHW], f32, tag="g")
                nc.gpsimd.tensor_tensor(gt[:, :], mk[:, :], g1b, op=OP.mult)
                nc.sync.dma_start(out=gd[bb, ch, :, :], in_=gt[:, :])

                # ---- numer = u - mk*(m2-m1) - nw;  q = numer / wnoise ----
                s2b = (s2c[:, :].rearrange("p (g k) -> p g k", g=NCH)
                       .unsqueeze(2).broadcast_to([128, NCH, C, NBLK]))
                t1 = wp.tile([128, CHW], f32, tag="t1")
                nc.gpsimd.tensor_tensor(t1[:, :], mk[:, :], s2b, op=OP.mult)
                numer = wp.tile([128, CHW], f32, tag="numer")
                nc.gpsimd.tensor_tensor(numer[:, :], un[:, :], t1[:, :],
                                        op=OP.subtract)
                qt = ep.tile([128, CHW], f32, tag=f"q{chunk}")
                nc.vector.tensor_tensor(qt[:, :], numer[:, :], rwT[:, :],
                                        op=OP.mult)
                qts.append((bb, ch, qt))

                # ---- erf group A anchored after chunk 5's gates ----
                if chunk == 5:
                    _, erf_a_last = _emit_erf_group(
                        qts[:NCHUNK // 2], t6_by_half[0] + t6_by_half[1])

            _emit_erf_group(qts[NCHUNK // 2:], t6_by_half[1])

    nc.compile()
    _fix_act_tables(nc, mybir)
    return nc


def _fix_act_tables(nc, mybir):
    """Retarget Exp/Ln/Copy activation-table loads to the combined exp+ln
    table and Erf loads to the erf-bearing table, then drop redundant
    reloads."""
    from concourse.hw_specs import get_activation_tables
    AFT = mybir.ActivationFunctionType
    tabs = list(get_activation_tables(nc.m.arch).items())
    targets = []
    for i, (_, fs) in enumerate(tabs):
        if AFT.Exp in fs and AFT.Ln in fs:
            targets.append((i, fs))
    for i, (_, fs) in enumerate(tabs):
        if AFT.Erf in fs:
            targets.append((i, fs))
    for blk in nc.m.functions[0].blocks:
        insts = blk.instructions
        loads = [(idx, inst) for idx, inst in enumerate(insts)
                 if isinstance(inst, mybir.InstLoadActFuncSet)]
        for li, (idx, load) in enumerate(loads):
            end = loads[li + 1][0] if li + 1 < len(loads) else len(insts)
            funcs = {i2.func for i2 in insts[idx + 1:end]
                     if isinstance(i2, mybir.InstActivation)}
            if not funcs:
                continue
            for tid, fs in targets:
                if funcs.issubset(fs):
                    load.act_func_set_id = tid
                    break
        cur = None
        to_remove = []
        for inst in insts:
            if isinstance(inst, mybir.InstLoadActFuncSet):
                if inst.act_func_set_id == cur and not inst.has_wait():
                    to_remove.append(inst)
                else:
                    cur = inst.act_func_set_id
            elif isinstance(inst, mybir.InstActivation):
                assert inst.func in tabs[cur][1], (inst.func, cur)
        for inst in to_remove:
            insts.remove(inst)


def make_in_maps(x, noise, wg_param, wnoise_param):
    identity = np.eye(128, dtype=np.float32)
    wgv = np.ascontiguousarray(wg_param, dtype=np.float32).reshape(C)
    wnv = np.ascontiguousarray(wnoise_param, dtype=np.float32).reshape(C)
    # per-partition scalars for p = c*8 + blk
    wgp = np.repeat(wgv, NBLK).reshape(128, 1).astype(np.float32)
    wnp = np.repeat(wnv, NBLK).reshape(128, 1).astype(np.float32)
    x = np.ascontiguousarray(x, dtype=np.float32).reshape(B, 128, FB)
    noise = np.ascontiguousarray(noise, dtype=np.float32).reshape(B, 128, FB)
    in_maps = []
    for i in range(NCORES):
        in_maps.append({"x": x[i * BPC:(i + 1) * BPC],
                        "noise": noise[i * BPC:(i + 1) * BPC],
                        "id_f": identity, "wgp": wgp, "wnp": wnp})
    return in_maps


def _decode_T(arr):
    """[BPC, CPB, 128, CHW] T-layout -> [BPC, C, H, W] standard layout.

    arr[bb, ch, pT, g*128 + c*8 + blk] = out[bb, c, blk*2048 + ch*512
                                             + g*128 + pT]
    """
    a = np.asarray(arr, dtype=np.float32).reshape(BPC, CPB, 128, NCH, C, NBLK)
    a = a.transpose(0, 4, 5, 1, 3, 2)  # [bb, c, blk, ch, g, pT]
    return a.reshape(BPC, C, H, W)


def kernel(x, noise, wg_param, wnoise_param):
    from concourse.bass_utils import run_bass_kernel_spmd

    if "nc" not in _CACHE:
        _CACHE["nc"] = _build()
    nc = _CACHE["nc"]
    in_maps = make_in_maps(x, noise, wg_param, wnoise_param)
    res = run_bass_kernel_spmd(nc, in_maps, list(range(NCORES)))
    G = np.empty((B, C, H, W), dtype=np.float32)
    L = np.empty((B, C, H, W), dtype=np.float32)
    for i in range(NCORES):
        G[i * BPC:(i + 1) * BPC] = _decode_T(res.results[i]["g_out"])
        L[i * BPC:(i + 1) * BPC] = _decode_T(res.results[i]["load_out"])
    return G, L
